# revision 20
# baseline (speedup 1.0000x reference)
"""Trainium2 Bass kernel for nn_Autotuner_FFN (dense MLP, 8-core data parallel).

Strategy (v2 — collapsed LayerNorm):
  * Host folds embeddings / op-linears / log2 scalings / LN mean-centerings
    into an effective W1 [186,1024] (ones-row carries the bias), exactly as
    v1. NEW: because LayerNorm is per-column scale-invariant and relu is
    positively homogeneous (and be1=be2=0, b2=0 for this model), the LN
    normalization never has to touch the activations:
        h1 = W1^T x            -> R1 = relu(h1)          (no LN1 at all)
        u  = W2c^T R1          -> sq = u^2, R2 = relu(u) (stats only)
        y  = (W3^T R2) * rsqrt(mean(u^2)+eps) + b3       (scale on [1,B] row)
    LN1's scale r1>0 cancels inside LN2; LN2's scale is applied to the single
    output row. g1/g2 are folded into W1/W2 columns on the host.
  * All GEMMs fp16 (1 cyc/row). The sum-of-squares stats GEMM runs in
    fp8e4m3 with MatmulPerfMode.DoubleRow (0.5 cyc/row, K=256 per matmul).
  * Layout: hidden on partitions, batch on free dim, 512-wide chunks.
    Emission is software-pipelined: PE does L1(c+1) between L2(c) and
    stats/L3(c) so it never waits on the Act/DVE relu/square passes.
  * Batch 65536 sharded 8192/core across 8 NeuronCores (pure DP).
"""
import numpy as np
import ml_dtypes

import concourse.bass as bass
import concourse.tile as tile
from concourse import bacc, mybir
from concourse.bass_utils import run_bass_kernel_spmd

AF = mybir.ActivationFunctionType
ALU = mybir.AluOpType
F32 = mybir.dt.float32
F16 = mybir.dt.float16
F8 = mybir.dt.float8e4
DR = mybir.MatmulPerfMode.DoubleRow

B = 65536
N_CORES = 8
B_CORE = B // N_CORES          # 8192
CH = 512                       # batch chunk (max moving dim)
NCH = B_CORE // CH             # 16
HID = 1024
MT = HID // 128                # 8 hidden m-tiles
EPS = 1e-5
LN2 = float(np.log(2.0))


# ---------------------------------------------------------------- host folds
def _fold_weights(inp):
    """Returns f16-packed weights. Folds one-hot tables, op linears, LN mean
    centering and g1/g2 affine gains into W1/W2. Requires be1=be2=0 and
    b2 constant (true for this model family)."""
    f8 = lambda x: np.asarray(x, np.float64)
    W1 = f8(inp["W1"]); b1 = f8(inp["b1"])
    emb_kc = f8(inp["emb_kc"]); emb_nl = f8(inp["emb_nl"])
    op_W = f8(inp["op_W"]); op_b = f8(inp["op_b"])
    emb_c = f8(inp["emb_contig"]); emb_s = f8(inp["emb_scalar"])
    emb_i = f8(inp["emb_indirect"])
    H = W1.shape[1]
    rows_A = []
    bias = b1.copy()
    rows_A.append(emb_kc @ W1[0:16])
    rows_A.append(emb_nl @ W1[16:32])
    W1_op = W1[32:944].reshape(57, 16, H)
    rows_A.append(np.einsum("ij,ijh->ih", op_W, W1_op))
    bias += np.einsum("ij,ijh->h", op_b, W1_op)
    rd_f2, rd_bool, rd_ss = [], [], []
    wd_f2, wd_bool, wd_ss = [], [], []
    for base, f2l, booll, ssl in ((947, rd_f2, rd_bool, rd_ss),
                                  (1027, wd_f2, wd_bool, wd_ss)):
        for d in range(4):
            Wd = W1[base + 20 * d: base + 20 * d + 20]
            f2l.append(Wd[0:2])
            ssl.append(Wd[2:8] / LN2)
            rows_b = []
            for e, sl in ((emb_c, slice(8, 12)), (emb_s, slice(12, 16)),
                          (emb_i, slice(16, 20))):
                rows_b.append((e[1] - e[0]) @ Wd[sl])
                bias += e[0] @ Wd[sl]
            booll.append(np.stack(rows_b))
    rows_A += [np.concatenate(rd_f2), np.concatenate(rd_bool),
               np.concatenate(wd_f2), np.concatenate(wd_bool),
               W1[1110:1112]]
    A = np.concatenate(rows_A)                               # [125, H]
    C = np.concatenate([W1[944:947] / LN2, W1[1107:1110] / LN2,
                        W1[1112:1115] / LN2,
                        np.concatenate(rd_ss), np.concatenate(wd_ss)])  # [57,H]
    W1_eff = np.concatenate([A, C])                          # [182, H]
    mu = np.concatenate([W1_eff, bias[None]], 0).mean(axis=1, keepdims=True)
    W1c = W1_eff - mu[:182]
    bc1 = bias - mu[182, 0]

    g1 = f8(inp["g1"]); be1 = f8(inp["be1"])
    g2 = f8(inp["g2"]); be2 = f8(inp["be2"])
    b2 = f8(inp["b2"])
    assert np.abs(be1).max() == 0.0 and np.abs(be2).max() == 0.0, \
        "collapsed-LN scheme needs be1=be2=0"
    bc2 = b2 - b2.mean()
    assert np.abs(bc2).max() < 1e-12, "collapsed-LN scheme needs constant b2"

    W1g = W1c * g1[None, :]
    bc1g = bc1 * g1
    W2 = f8(inp["W2"])
    W2g = (W2 - W2.mean(axis=1, keepdims=True)) * g2[None, :]

    # pack W1 into two 128-row k-tiles: t0 = A rows 0..124 + bias row at 125,
    # t1 = C rows 0..56, rest zero.
    W1p = np.zeros((128, 2, H), np.float64)
    W1p[0:125, 0] = W1g[0:125]
    W1p[125, 0] = bc1g
    W1p[0:57, 1] = W1g[125:182]
    W2p = np.ascontiguousarray(
        W2g.reshape(MT, 128, H).transpose(1, 0, 2))          # [128, MT, H]
    W3 = f8(inp["W3"])[:, 0]
    w3p = np.ascontiguousarray(W3.reshape(MT, 128).T)        # [128, MT]
    # stats lhsT: DR weight APs need 16B-aligned strides between k-tiles, so
    # spread the per-k values 16 columns apart: value for k-tile k at [:, k*16]
    g2i2 = np.zeros((128, MT, 16), np.float64)
    g2i2[:, :, 0] = (1.0 / (g2 * g2)).reshape(MT, 128).T
    b3 = float(np.asarray(inp["b3"], np.float64)[0])
    return (W1p.reshape(128, 2 * H).astype(np.float16),
            W2p.reshape(128, MT * H).astype(np.float16),
            w3p.astype(np.float16),
            g2i2.reshape(128, MT * 16).astype(ml_dtypes.float8_e4m3),
            b3)


def _build_xt(inp):
    """Feature matrix [128, 2, B] f16: k-tile 0 = raw features + ones row,
    k-tile 1 = sign*ln(|x|+1)-transformed features (rows 0..56)."""
    Bn = inp["op_vec"].shape[0]
    kc = np.asarray(inp["kernel_category_idx"]).astype(np.int64)
    nl = np.asarray(inp["num_of_loops_idx"]).astype(np.int64)
    f = lambda k: np.asarray(inp[k], np.float32)
    XT = np.zeros((128, 2, Bn), np.float32)
    X0 = XT[:, 0]
    X0[0:10] = (np.arange(10)[:, None] == kc[None, :])
    X0[10:26] = (np.arange(16)[:, None] == nl[None, :])
    X0[26:83] = f("op_vec").T
    X0[83:91] = f("read_dep_float")[:, :, 0:2].reshape(Bn, 8).T
    X0[91:103] = np.asarray(inp["read_dep_bools"]).reshape(Bn, 12).T
    X0[103:111] = f("write_dep_float")[:, :, 0:2].reshape(Bn, 8).T
    X0[111:123] = np.asarray(inp["write_dep_bools"]).reshape(Bn, 12).T
    X0[123:125] = f("rest_vec")[:, 3:5].T
    X0[125] = 1.0
    raw = np.concatenate([
        f("size_hints").T,
        f("rest_vec")[:, [0, 1, 2, 5, 6, 7]].T,
        f("read_dep_float")[:, :, 2:8].reshape(Bn, 24).T,
        f("write_dep_float")[:, :, 2:8].reshape(Bn, 24).T], 0)  # [57, B]
    XT[0:57, 1] = np.sign(raw) * np.log(np.abs(raw) + 1.0)
    return XT.astype(np.float16)


# ---------------------------------------------------------------- device prog
DEFAULT_CFG = dict(xin_bufs=3, r1_bufs=2, sq_bufs=2, r2_bufs=2,
                   ps_l1_bufs=3, ps_l2_bufs=3, ps_s_bufs=1, ps_v_bufs=1,
                   small_bufs=4, sq_fp8=True,
                   relu1_eng="dve", sq_eng="act", relu2_eng="act")


def build_program(loop_iters=None, cfg=None, has_b3=False):
    cfg = {**DEFAULT_CFG, **(cfg or {})}
    nc = bacc.Bacc("TRN2", target_bir_lowering=False, debug=False)
    xt = nc.dram_tensor("xt", [128, 2 * B_CORE], F16, kind="ExternalInput")
    w1 = nc.dram_tensor("w1", [128, 2 * HID], F16, kind="ExternalInput")
    w2 = nc.dram_tensor("w2", [128, MT * HID], F16, kind="ExternalInput")
    w3p = nc.dram_tensor("w3p", [128, MT], F16, kind="ExternalInput")
    g2i2 = nc.dram_tensor("g2i2", [128, MT * 16], F8, kind="ExternalInput")
    b3t = nc.dram_tensor("b3t", [1, 1], F32, kind="ExternalInput")
    y = nc.dram_tensor("y", [1, B_CORE], F32, kind="ExternalOutput")

    SQDT = F8 if cfg["sq_fp8"] else F16

    from contextlib import ExitStack
    with tile.TileContext(nc) as tc, ExitStack() as ctx, \
            nc.allow_low_precision(reason="fp16/fp8 rounding is intentional"):
        const = ctx.enter_context(tc.tile_pool(name="const", bufs=1))
        xin = ctx.enter_context(tc.tile_pool(name="xin", bufs=cfg["xin_bufs"]))
        R1p = ctx.enter_context(tc.tile_pool(name="R1p", bufs=cfg["r1_bufs"]))
        SQp = ctx.enter_context(tc.tile_pool(name="SQp", bufs=cfg["sq_bufs"]))
        R2p = ctx.enter_context(tc.tile_pool(name="R2p", bufs=cfg["r2_bufs"]))
        small = ctx.enter_context(tc.tile_pool(name="small", bufs=cfg["small_bufs"]))
        ps_l1 = ctx.enter_context(tc.tile_pool(name="ps_l1", bufs=cfg["ps_l1_bufs"], space="PSUM"))
        ps_l2 = ctx.enter_context(tc.tile_pool(name="ps_l2", bufs=cfg["ps_l2_bufs"], space="PSUM"))
        ps_s = ctx.enter_context(tc.tile_pool(name="ps_s", bufs=cfg["ps_s_bufs"], space="PSUM"))
        ps_v = ctx.enter_context(tc.tile_pool(name="ps_v", bufs=cfg["ps_v_bufs"], space="PSUM"))

        # ---- one-time setup
        w1t = const.tile([128, 2 * HID], F16, tag="w1t")
        nc.sync.dma_start(w1t[:], w1.ap())
        w2t = const.tile([128, MT * HID], F16, tag="w2t")
        nc.sync.dma_start(w2t[:], w2.ap())
        w3t = const.tile([128, MT], F16, tag="w3t")
        nc.sync.dma_start(w3t[:], w3p.ap())
        g2t = const.tile([128, MT * 16], F8, tag="g2t")
        nc.sync.dma_start(g2t[:], g2i2.ap())
        b3s = const.tile([1, 1], F32, tag="b3s")
        nc.sync.dma_start(b3s[:], b3t.ap())
        eps_t = const.tile([1, 1], F32, tag="eps_t")
        nc.vector.memset(eps_t[:], EPS)
        if not cfg["sq_fp8"]:
            ones_t = const.tile([128, MT], F16, tag="ones_t")
            nc.vector.memset(ones_t[:], 1.0)

        def eng(name):
            return {"dve": nc.vector, "act": nc.scalar, "pool": nc.gpsimd}[name]

        def relu_to(engine_name, dst, src):
            if engine_name == "act":
                nc.scalar.activation(dst, src, AF.Relu)
            else:
                eng(engine_name).tensor_scalar_max(dst, src, 0.0)

        def square_to(engine_name, dst, src):
            if engine_name == "act":
                nc.scalar.activation(dst, src, AF.Square)
            else:
                eng(engine_name).tensor_mul(dst, src, src)

        ablate = cfg.get("ablate", "full")
        if ablate != "full":
            R1const = const.tile([128, MT * CH], F16, tag="R1const")
            nc.vector.memset(R1const[:], 0.01)
            yconst = const.tile([1, CH], F32, tag="yconst")
            nc.vector.memset(yconst[:], 0.0)

        def emit_l1(c):
            if ablate == "pe_nodma":
                x = None
            else:
                x = xin.tile([128, 2 * CH], F16, tag="x")
                nc.sync.dma_start(x[:, 0:CH], xt.ap()[:, c * CH:(c + 1) * CH])
                nc.sync.dma_start(x[:, CH:2 * CH],
                                  xt.ap()[:, B_CORE + c * CH:B_CORE + (c + 1) * CH])
            R1 = R1p.tile([128, MT * CH], F16, tag="R1")
            for m in range(MT):
                p1 = ps_l1.tile([128, CH], F32, tag="p1")
                if ablate == "pe_nodma":
                    nc.tensor.matmul(p1[:], w1t[:, m * 128:(m + 1) * 128],
                                     R1const[:, 0:CH], start=True, stop=False)
                    nc.tensor.matmul(p1[:], w1t[:, HID + m * 128:HID + (m + 1) * 128],
                                     R1const[:, CH:2 * CH], start=False, stop=True)
                    continue
                nc.tensor.matmul(p1[:], w1t[:, m * 128:(m + 1) * 128],
                                 x[:, 0:CH], start=True, stop=False)
                nc.tensor.matmul(p1[:], w1t[:, HID + m * 128:HID + (m + 1) * 128],
                                 x[:, CH:2 * CH], start=False, stop=True)
                if ablate in ("pe_only", "pe_nodma"):
                    continue
                e = cfg["relu1_eng"]
                if e == "alt":
                    e = "dve" if m % 2 == 0 else "act"
                relu_to(e, R1[:, m * CH:(m + 1) * CH], p1[:])
            return R1const if ablate in ("pe_only", "pe_nodma") else R1

        def emit_l2(R1):
            SQ = SQp.tile([128, MT * CH], SQDT, tag="SQ")
            R2 = R2p.tile([128, MT * CH], F16, tag="R2")
            for m in range(MT):
                p2 = ps_l2.tile([128, CH], F32, tag="p2")
                for k in range(MT):
                    nc.tensor.matmul(p2[:], w2t[:, k * HID + m * 128:k * HID + (m + 1) * 128],
                                     R1[:, k * CH:(k + 1) * CH],
                                     start=(k == 0), stop=(k == MT - 1))
                if ablate in ("pe_only", "pe_nodma"):
                    continue
                se, re = cfg["sq_eng"], cfg["relu2_eng"]
                if se == "alt":
                    se = "act" if m % 2 == 0 else "dve"
                if re == "alt":
                    re = "dve" if m % 2 == 0 else "act"
                square_to(se, SQ[:, m * CH:(m + 1) * CH], p2[:])
                relu_to(re, R2[:, m * CH:(m + 1) * CH], p2[:])
            return SQ, R2

        def emit_stats_l3(c, SQ, R2):
            if ablate in ("pe_only", "pe_nodma"):
                nc.sync.dma_start(y.ap()[0:1, c * CH:(c + 1) * CH], yconst[:])
                return
            ps = ps_s.tile([1, CH], F32, tag="ps")
            if cfg["sq_fp8"]:
                sq3 = SQ[:].rearrange("p (t n) -> p t n", t=MT)
                g23 = g2t[:].rearrange("p (k s) -> p k s", k=MT)
                for k2 in range(MT // 2):
                    nc.tensor.matmul(ps[:], g23[:, 2 * k2:2 * k2 + 2, 0:1],
                                     sq3[:, 2 * k2:2 * k2 + 2, :],
                                     start=(k2 == 0), stop=(k2 == MT // 2 - 1),
                                     perf_mode=DR)
            else:
                for k in range(MT):
                    nc.tensor.matmul(ps[:], ones_t[:, k:k + 1],
                                     SQ[:, k * CH:(k + 1) * CH],
                                     start=(k == 0), stop=(k == MT - 1))
            pv = ps_v.tile([1, CH], F32, tag="pv")
            for k in range(MT):
                nc.tensor.matmul(pv[:], w3t[:, k:k + 1],
                                 R2[:, k * CH:(k + 1) * CH],
                                 start=(k == 0), stop=(k == MT - 1))
            sd = small.tile([1, CH], F32, tag="sd")
            nc.scalar.activation(sd[:], ps[:], AF.Sqrt,
                                 bias=eps_t[:], scale=1.0 / HID)
            rs = small.tile([1, CH], F32, tag="rs")
            nc.vector.reciprocal(rs[:], sd[:])
            yt = small.tile([1, CH], F32, tag="yt")
            nc.vector.tensor_mul(yt[:], pv[:], rs[:])
            if has_b3:
                yb = small.tile([1, CH], F32, tag="yb")
                nc.scalar.activation(yb[:], yt[:], AF.Identity, bias=b3s[:])
                yt = yb
            nc.sync.dma_start(y.ap()[0:1, c * CH:(c + 1) * CH], yt[:])

        def body():
            prev = None
            for c in range(NCH):
                R1 = emit_l1(c)
                if prev is not None:
                    emit_stats_l3(prev[0], prev[1], prev[2])
                SQ, R2 = emit_l2(R1)
                prev = (c, SQ, R2)
            emit_stats_l3(prev[0], prev[1], prev[2])

        if loop_iters is None:
            body()
        else:
            with tc.For_i(0, loop_iters, 1):
                body()
    nc.compile()
    return nc


# ---------------------------------------------------------------- entry point
_CACHE = {}

BEST_CFG = dict(sq_fp8=True, relu1_eng="alt", sq_eng="act", relu2_eng="dve", ps_l1_bufs=4, ps_l2_bufs=2)


def _get_program(has_b3):
    key = ("prog", has_b3)
    if key not in _CACHE:
        _CACHE[key] = build_program(cfg=BEST_CFG, has_b3=has_b3)
    return _CACHE[key]


def make_in_maps(inputs):
    inp = {k: np.asarray(v) for k, v in inputs.items()}
    W1p, W2p, w3p, g2i2, b3 = _fold_weights(inp)
    XT = _build_xt(inp)
    shared = {
        "w1": W1p, "w2": W2p, "w3p": w3p, "g2i2": g2i2,
        "b3t": np.full((1, 1), b3, np.float32),
    }
    has_b3 = b3 != 0.0
    in_maps = []
    for c in range(N_CORES):
        m = dict(shared)
        m["xt"] = np.ascontiguousarray(
            XT[:, :, c * B_CORE:(c + 1) * B_CORE].reshape(128, 2 * B_CORE))
        in_maps.append(m)
    return in_maps, has_b3


def kernel(**inputs) -> np.ndarray:
    in_maps, has_b3 = make_in_maps(inputs)
    nc = _get_program(has_b3)
    res = run_bass_kernel_spmd(nc, in_maps, core_ids=list(range(N_CORES)))
    y = np.concatenate([r["y"][0] for r in res.results])
    return y.reshape(B, 1).astype(np.float32)


if __name__ == "__main__":
    import jax
    import reference
    cpu = jax.devices("cpu")[0]
    with jax.default_device(cpu):
        inp = reference.setup_inputs()
        ref = np.asarray(reference.reference(**inp))
    out = kernel(**{k: np.asarray(v) for k, v in inp.items()})
    err = np.abs(out - ref)
    scale = np.abs(ref).max()
    print("max_abs", err.max(), "rel(vs scale)", err.max() / scale,
          "mean_rel", (err / (np.abs(ref) + 1e-6)).mean())
